# revision 3
# baseline (speedup 1.0000x reference)
"""GIDD loss kernel for Trainium2 (8 NeuronCores, token-parallel).

Math: with gamma=1 the q_t distribution takes only 3 distinct values per row
(at input_id, at MASK, elsewhere), so the vocab-wide KL reduces to two row
statistics computed on device:
    Z  = sum_{v != MASK} exp(logit_v)                  (softmax denominator)
    S  = sum_{v != MASK} log(exp(logit_v) + D),  D = (c_t/alpha_hat) * Z
Everything else is O(B*T) and is evaluated on the host while unsharding.

Device kernel per core: 256 tokens (2 tiles x 128 partitions), vocab on the
free dim. Pass 1: DMA logits chunks, ACT exp with fused accum (row sum -> Z).
exp values cached in SBUF as bf16. Pass 2: ACT ln(E + D) with per-partition
bias and fused accum (-> S). All tokens of one core belong to one sample, so
the single per-core scalar r = c_t/alpha_hat arrives as a tiny input tensor.
"""

import numpy as np

VOCAB = 32000
MASK_ID = 31999
NV = VOCAB - 1  # vocab columns participating in softmax/KL (mask col excluded)
B, T = 2, 1024
N_TOK = B * T
N_CORES = 8
TOK_PER_CORE = N_TOK // N_CORES  # 256
P = 128
N_TILES = TOK_PER_CORE // P  # 2
CHUNK = 3200
LOG_B = -11.0

_CACHE = {}


def _ensure_ntff_hook():
    """Provide antenv.axon_hooks (absent on this image) and install the
    NTFF profile hook so trace=True can report HW exec time."""
    import sys
    import types

    if "antenv.axon_hooks" not in sys.modules:
        mod = types.ModuleType("antenv.axon_hooks")
        mod._hook = None

        def set_axon_ntff_profile_hook(h, _mod=mod):
            _mod._hook = h

        def get_axon_ntff_profile_hook(_mod=mod):
            return _mod._hook

        mod.set_axon_ntff_profile_hook = set_axon_ntff_profile_hook
        mod.get_axon_ntff_profile_hook = get_axon_ntff_profile_hook
        sys.modules["antenv.axon_hooks"] = mod
        try:
            import antenv

            antenv.axon_hooks = mod
        except ImportError:
            pass
    try:
        from antenv.axon_hooks import (
            get_axon_ntff_profile_hook,
            set_axon_ntff_profile_hook,
        )

        if get_axon_ntff_profile_hook() is None:
            from trn_agent_boot.trn_boot import _ntff_profile_via_ctypes

            hook = _ntff_profile_via_ctypes("/opt/axon/libaxon_pjrt.so")
            if hook is not None:
                set_axon_ntff_profile_hook(hook)
    except Exception:
        pass


def _chunks():
    offs = []
    o = 0
    while o < NV:
        offs.append((o, min(CHUNK, NV - o)))
        o += CHUNK
    return offs


def _build():
    import concourse.tile as tile
    from concourse import bacc, mybir

    nc = bacc.Bacc("TRN2", target_bir_lowering=False, debug=False,
                   num_devices=N_CORES)
    xt = nc.dram_tensor("xt", [TOK_PER_CORE, VOCAB], mybir.dt.float32,
                        kind="ExternalInput")
    rconst = nc.dram_tensor("rconst", [P, 1], mybir.dt.float32,
                            kind="ExternalInput")
    out = nc.dram_tensor("out", [P, 2 * N_TILES], mybir.dt.float32,
                         kind="ExternalOutput")

    offs = _chunks()
    nch = len(offs)
    f32 = mybir.dt.float32
    bf16 = mybir.dt.bfloat16
    EXP = mybir.ActivationFunctionType.Exp
    LN = mybir.ActivationFunctionType.Ln

    with tile.TileContext(nc) as tc:
        with (
            tc.tile_pool(name="land", bufs=3) as land_pool,
            tc.tile_pool(name="epool", bufs=N_TILES) as epool,
            tc.tile_pool(name="scr", bufs=2) as scr_pool,
            tc.tile_pool(name="small", bufs=2 * N_TILES) as small,
            tc.tile_pool(name="singles", bufs=1) as singles,
        ):
            rt = singles.tile([P, 1], f32)
            nc.sync.dma_start(out=rt, in_=rconst[:, :])

            for j in range(N_TILES):
                E = epool.tile([P, NV], bf16, tag="E")
                zparts = small.tile([P, nch], f32, tag="zparts")
                r0 = j * P
                for k, (off, cs) in enumerate(offs):
                    land = land_pool.tile([P, CHUNK], f32, tag="land")
                    nc.sync.dma_start(
                        out=land[:, :cs],
                        in_=xt[r0:r0 + P, off:off + cs],
                    )
                    nc.scalar.activation(
                        out=E[:, off:off + cs],
                        in_=land[:, :cs],
                        func=EXP,
                        accum_out=zparts[:, k:k + 1],
                    )
                Z1 = small.tile([P, 1], f32, tag="Z1")
                nc.vector.reduce_sum(Z1, zparts, axis=mybir.AxisListType.X)
                D = small.tile([P, 1], f32, tag="D")
                nc.vector.tensor_mul(D, Z1, rt)

                sparts = small.tile([P, nch], f32, tag="sparts")
                for k, (off, cs) in enumerate(offs):
                    scr = scr_pool.tile([P, CHUNK], bf16, tag="scr")
                    nc.scalar.activation(
                        out=scr[:, :cs],
                        in_=E[:, off:off + cs],
                        func=LN,
                        bias=D,
                        accum_out=sparts[:, k:k + 1],
                    )
                S1 = small.tile([P, 1], f32, tag="S1")
                nc.vector.reduce_sum(S1, sparts, axis=mybir.AxisListType.X)

                st = small.tile([P, 2], f32, tag="st")
                nc.vector.tensor_copy(st[:, 0:1], Z1)
                nc.vector.tensor_copy(st[:, 1:2], S1)
                nc.sync.dma_start(out=out[:, 2 * j:2 * j + 2], in_=st)
    nc.compile()
    return nc


def _get_nc():
    if "nc" not in _CACHE:
        _CACHE["nc"] = _build()
    return _CACHE["nc"]


def _run_device(logits_flat, r_per_core, trace=False):
    _ensure_ntff_hook()
    from concourse.bass_utils import run_bass_kernel_spmd

    nc = _get_nc()
    in_maps = []
    for c in range(N_CORES):
        sl = logits_flat[c * TOK_PER_CORE:(c + 1) * TOK_PER_CORE]
        in_maps.append({
            "xt": np.ascontiguousarray(sl),
            "rconst": np.full((P, 1), r_per_core[c], dtype=np.float32),
        })
    res = run_bass_kernel_spmd(nc, in_maps, list(range(N_CORES)), trace=trace)
    Z1 = np.empty(N_TOK, np.float64)
    S1 = np.empty(N_TOK, np.float64)
    for c in range(N_CORES):
        o = res.results[c]["out"]  # [P, 2*N_TILES]
        for j in range(N_TILES):
            t0 = c * TOK_PER_CORE + j * P
            Z1[t0:t0 + P] = o[:, 2 * j].astype(np.float64)
            S1[t0:t0 + P] = o[:, 2 * j + 1].astype(np.float64)
    return Z1.reshape(B, T), S1.reshape(B, T), res


def kernel(logits, input_ids, attention_mask, z_t, t, _trace=False):
    logits = np.asarray(logits)
    input_ids = np.asarray(input_ids)
    attention_mask = np.asarray(attention_mask)
    z_t = np.asarray(z_t)
    t = np.asarray(t)

    # ---- per-sample schedule scalars (float64 host math) ----
    t64 = t.astype(np.float64)  # [B]
    B_ = np.exp(np.float64(LOG_B))
    c = np.sqrt(t64) * np.sqrt(1.0 - t64) * B_
    C = 1.0 + (VOCAB - 2) * c
    alpha_hat = (1.0 - t64) - c
    c_p = 0.5 * (1.0 - 2.0 * t64) / (t64 * (1.0 - t64)) * c
    C_p = (VOCAB - 2) * c_p
    alpha_hat_p = -1.0 - c_p

    # ---- device: Z and S row statistics over the vocab ----
    r_tok = (c / alpha_hat)  # [B]
    core_b = (np.arange(N_CORES) * TOK_PER_CORE) // T
    r_per_core = r_tok[core_b].astype(np.float32)
    Z1, S1, res = _run_device(
        np.ascontiguousarray(logits.reshape(N_TOK, VOCAB)), r_per_core,
        trace=_trace)

    # ---- host: per-token weights ----
    tb = t64[:, None]  # [B,1]
    cb, Cb, ab = c[:, None], C[:, None], alpha_hat[:, None]
    cpb, Cpb, apb = c_p[:, None], C_p[:, None], alpha_hat_p[:, None]
    is_mask = (z_t == MASK_ID).astype(np.float64)
    pi_hat = tb * is_mask + cb * (1.0 - is_mask)
    pi_hat_p = 1.0 * is_mask + cpb * (1.0 - is_mask)
    alpha = ab / Cb
    pi_beta = pi_hat / Cb
    alpha_ratio = apb / ab - Cpb / Cb  # [B,1]
    omega = (pi_hat_p - apb / ab * pi_hat) / Cb
    is_x = (z_t == input_ids).astype(np.float64)
    elbo_w = (1.0 - is_x) * (omega / pi_beta) + is_x * (omega / (alpha + pi_beta))
    ws = np.clip(elbo_w, 0.0, 100.0)

    # ---- host: assemble KL + correction from device stats ----
    la = np.log(alpha_hat)[:, None]
    logC = np.log(C)[:, None]
    logZ = np.log(Z1)  # [B,T]
    D = (c / alpha_hat)[:, None] * Z1
    # log p_t[v != MASK] = la + log(exp(l_v) + D) - logZ - logC
    sumLP_nm = NV * (la - logZ - logC) + S1
    l_x = np.take_along_axis(logits, input_ids[..., None], axis=2)[..., 0]
    l_x = l_x.astype(np.float64)
    LPx = la + np.log(np.exp(l_x) + D) - logZ - logC
    LPm = np.log(tb) - logC
    qc = cb / Cb
    qx = (ab + cb) / Cb
    qm = tb / Cb
    T1 = (VOCAB - 2) * qc * np.log(qc) + qx * np.log(qx) + qm * np.log(qm)
    T2 = qc * (sumLP_nm - LPx) + qx * LPx + qm * LPm
    kl = T1 - T2

    l_z = np.take_along_axis(logits, (z_t % VOCAB)[..., None], axis=2)[..., 0]
    l_z = l_z.astype(np.float64)
    LPz = np.where(is_mask > 0.5, LPm, la + np.log(np.exp(l_z) + D) - logZ - logC)
    LQz = np.where(is_mask > 0.5, np.log(qm),
                   np.where(is_x > 0.5, np.log(qx), np.log(qc)))
    lr = LQz - LPz
    corr = -lr + np.exp(lr)
    lt = kl + corr

    mask = attention_mask.astype(np.float64)
    elbo = (elbo_w * lt + alpha_ratio).astype(np.float32)
    loss = np.float32((ws * lt * mask).sum() / mask.sum())
    if _trace:
        return (loss, elbo), res
    return (loss, elbo)


# revision 11
# speedup vs baseline: 1.3432x; 1.3432x over previous
"""GIDD loss kernel for Trainium2 (8 NeuronCores, token-parallel).

Math: with gamma=1 the q_t distribution takes only 3 distinct values per row
(at input_id, at MASK, elsewhere), so the vocab-wide KL reduces to two row
statistics computed on device:
    Z  = sum_{v != MASK} exp(logit_v)                  (softmax denominator)
    S  = sum_{v != MASK} log(exp(logit_v) + D),  D = (c_t/alpha_hat) * Z
Everything else is O(B*T) and is evaluated on the host while unsharding.

Device kernel per core: 256 tokens (2 tiles x 128 partitions), vocab on the
free dim. Pass 1: DMA logits chunks, ACT exp with fused accum (row sum -> Z).
exp values cached in SBUF as bf16. Pass 2: ACT ln(E + D) with per-partition
bias and fused accum (-> S). All tokens of one core belong to one sample, so
the single per-core scalar r = c_t/alpha_hat arrives as a tiny input tensor.
"""

import numpy as np

VOCAB = 32000
MASK_ID = 31999
NV = VOCAB - 1  # vocab columns participating in softmax/KL (mask col excluded)
B, T = 2, 1024
N_TOK = B * T
N_CORES = 8
TOK_PER_CORE = N_TOK // N_CORES  # 256
P = 128
N_TILES = TOK_PER_CORE // P  # 2
CHUNK = 6400
LAND_BUFS = 4
LOG_B = -11.0

_CACHE = {}


def _ensure_ntff_hook():
    """Provide antenv.axon_hooks (absent on this image) and install the
    NTFF profile hook so trace=True can report HW exec time."""
    import sys
    import types

    if "antenv.axon_hooks" not in sys.modules:
        mod = types.ModuleType("antenv.axon_hooks")
        mod._hook = None

        def set_axon_ntff_profile_hook(h, _mod=mod):
            _mod._hook = h

        def get_axon_ntff_profile_hook(_mod=mod):
            return _mod._hook

        mod.set_axon_ntff_profile_hook = set_axon_ntff_profile_hook
        mod.get_axon_ntff_profile_hook = get_axon_ntff_profile_hook
        sys.modules["antenv.axon_hooks"] = mod
        try:
            import antenv

            antenv.axon_hooks = mod
        except ImportError:
            pass
    try:
        from antenv.axon_hooks import (
            get_axon_ntff_profile_hook,
            set_axon_ntff_profile_hook,
        )

        if get_axon_ntff_profile_hook() is None:
            from trn_agent_boot.trn_boot import _ntff_profile_via_ctypes

            hook = _ntff_profile_via_ctypes("/opt/axon/libaxon_pjrt.so")
            if hook is not None:
                set_axon_ntff_profile_hook(hook)
    except Exception:
        pass


def _chunks():
    offs = []
    o = 0
    while o < NV:
        offs.append((o, min(CHUNK, NV - o)))
        o += CHUNK
    return offs


def _force_single_act_table(bacc_mod):
    """Make the act-table pass satisfy Exp/Ln only via the combined
    natural_log_exp_and_others set, so the kernel loads one table once
    instead of ping-ponging between the exp-only and ln-only sets."""
    import bass_rust
    from concourse import mybir
    from concourse.hw_specs import get_activation_tables

    def patched(self):
        has_activation = any(
            isinstance(i, mybir.InstActivation)
            for b in self.main_func.blocks
            for i in b.instructions
        )
        if not has_activation:
            return
        both = {mybir.ActivationFunctionType.Exp,
                mybir.ActivationFunctionType.Ln}
        tables = []
        for name, funcs in get_activation_tables(self.m.arch).items():
            if name != "natural_log_exp_and_others":
                funcs = set(funcs) - both
            tables.append((name, funcs))
        bass_rust.insert_act_table_loads(self, tables)

    return patched


def _build():
    import concourse.tile as tile
    from concourse import bacc, mybir

    nc = bacc.Bacc("TRN2", target_bir_lowering=False, debug=False,
                   num_devices=N_CORES)
    nc.insert_act_table_loads = _force_single_act_table(bacc).__get__(nc)
    xt = nc.dram_tensor("xt", [TOK_PER_CORE, VOCAB], mybir.dt.float16,
                        kind="ExternalInput")
    rconst = nc.dram_tensor("rconst", [P, 1], mybir.dt.float32,
                            kind="ExternalInput")
    out = nc.dram_tensor("out", [P, 2 * N_TILES], mybir.dt.float32,
                         kind="ExternalOutput")

    offs = _chunks()
    nch = len(offs)
    f32 = mybir.dt.float32
    f16 = mybir.dt.float16
    EXP = mybir.ActivationFunctionType.Exp
    LN = mybir.ActivationFunctionType.Ln

    with tile.TileContext(nc) as tc:
        with (
            tc.tile_pool(name="land", bufs=LAND_BUFS) as land_pool,
            tc.tile_pool(name="epool", bufs=N_TILES) as epool,
            tc.tile_pool(name="small", bufs=N_TILES) as small,
            tc.tile_pool(name="singles", bufs=1) as singles,
        ):
            rt = singles.tile([P, 1], f32)
            nc.sync.dma_start(out=rt, in_=rconst[:, :])

            for j in range(N_TILES):
                E = epool.tile([P, NV], f16, tag="E")
                zparts = small.tile([P, nch], f32, tag="zparts")
                r0 = j * P
                for k, (off, cs) in enumerate(offs):
                    land = land_pool.tile([P, CHUNK], f16, tag="land")
                    nc.sync.dma_start(
                        out=land[:, :cs],
                        in_=xt[r0:r0 + P, off:off + cs],
                    )
                    nc.scalar.activation(
                        out=E[:, off:off + cs],
                        in_=land[:, :cs],
                        func=EXP,
                        accum_out=zparts[:, k:k + 1],
                    )
                Z1 = small.tile([P, 1], f32, tag="Z1")
                nc.vector.reduce_sum(Z1, zparts, axis=mybir.AxisListType.X)
                D = small.tile([P, 1], f32, tag="D")
                nc.vector.tensor_mul(D, Z1, rt)

                # single whole-row ln pass, in place over E, sum via accum
                S1 = small.tile([P, 1], f32, tag="S1")
                nc.scalar.activation(
                    out=E[:, :],
                    in_=E[:, :],
                    func=LN,
                    bias=D,
                    accum_out=S1,
                )

                st = small.tile([P, 2], f32, tag="st")
                nc.vector.tensor_copy(st[:, 0:1], Z1)
                nc.vector.tensor_copy(st[:, 1:2], S1)
                nc.sync.dma_start(out=out[:, 2 * j:2 * j + 2], in_=st)
    nc.compile()
    return nc


def _get_nc():
    if "nc" not in _CACHE:
        _CACHE["nc"] = _build()
    return _CACHE["nc"]


def _run_device(logits_flat, r_per_core, trace=False):
    _ensure_ntff_hook()
    from concourse.bass_utils import run_bass_kernel_spmd

    nc = _get_nc()
    in_maps = []
    for c in range(N_CORES):
        sl = logits_flat[c * TOK_PER_CORE:(c + 1) * TOK_PER_CORE]
        in_maps.append({
            "xt": np.ascontiguousarray(sl, dtype=np.float16),
            "rconst": np.full((P, 1), r_per_core[c], dtype=np.float32),
        })
    res = run_bass_kernel_spmd(nc, in_maps, list(range(N_CORES)), trace=trace)
    Z1 = np.empty(N_TOK, np.float64)
    S1 = np.empty(N_TOK, np.float64)
    for c in range(N_CORES):
        o = res.results[c]["out"]  # [P, 2*N_TILES]
        for j in range(N_TILES):
            t0 = c * TOK_PER_CORE + j * P
            Z1[t0:t0 + P] = o[:, 2 * j].astype(np.float64)
            S1[t0:t0 + P] = o[:, 2 * j + 1].astype(np.float64)
    return Z1.reshape(B, T), S1.reshape(B, T), res


def kernel(logits, input_ids, attention_mask, z_t, t, _trace=False):
    logits = np.asarray(logits)
    input_ids = np.asarray(input_ids)
    attention_mask = np.asarray(attention_mask)
    z_t = np.asarray(z_t)
    t = np.asarray(t)

    # ---- per-sample schedule scalars (float64 host math) ----
    t64 = t.astype(np.float64)  # [B]
    B_ = np.exp(np.float64(LOG_B))
    c = np.sqrt(t64) * np.sqrt(1.0 - t64) * B_
    C = 1.0 + (VOCAB - 2) * c
    alpha_hat = (1.0 - t64) - c
    c_p = 0.5 * (1.0 - 2.0 * t64) / (t64 * (1.0 - t64)) * c
    C_p = (VOCAB - 2) * c_p
    alpha_hat_p = -1.0 - c_p

    # ---- device: Z and S row statistics over the vocab ----
    r_tok = (c / alpha_hat)  # [B]
    core_b = (np.arange(N_CORES) * TOK_PER_CORE) // T
    r_per_core = r_tok[core_b].astype(np.float32)
    Z1, S1, res = _run_device(
        np.ascontiguousarray(logits.reshape(N_TOK, VOCAB)), r_per_core,
        trace=_trace)

    # ---- host: per-token weights ----
    tb = t64[:, None]  # [B,1]
    cb, Cb, ab = c[:, None], C[:, None], alpha_hat[:, None]
    cpb, Cpb, apb = c_p[:, None], C_p[:, None], alpha_hat_p[:, None]
    is_mask = (z_t == MASK_ID).astype(np.float64)
    pi_hat = tb * is_mask + cb * (1.0 - is_mask)
    pi_hat_p = 1.0 * is_mask + cpb * (1.0 - is_mask)
    alpha = ab / Cb
    pi_beta = pi_hat / Cb
    alpha_ratio = apb / ab - Cpb / Cb  # [B,1]
    omega = (pi_hat_p - apb / ab * pi_hat) / Cb
    is_x = (z_t == input_ids).astype(np.float64)
    elbo_w = (1.0 - is_x) * (omega / pi_beta) + is_x * (omega / (alpha + pi_beta))
    ws = np.clip(elbo_w, 0.0, 100.0)

    # ---- host: assemble KL + correction from device stats ----
    la = np.log(alpha_hat)[:, None]
    logC = np.log(C)[:, None]
    logZ = np.log(Z1)  # [B,T]
    D = (c / alpha_hat)[:, None] * Z1
    # log p_t[v != MASK] = la + log(exp(l_v) + D) - logZ - logC
    sumLP_nm = NV * (la - logZ - logC) + S1
    l_x = np.take_along_axis(logits, input_ids[..., None], axis=2)[..., 0]
    l_x = l_x.astype(np.float64)
    LPx = la + np.log(np.exp(l_x) + D) - logZ - logC
    LPm = np.log(tb) - logC
    qc = cb / Cb
    qx = (ab + cb) / Cb
    qm = tb / Cb
    T1 = (VOCAB - 2) * qc * np.log(qc) + qx * np.log(qx) + qm * np.log(qm)
    T2 = qc * (sumLP_nm - LPx) + qx * LPx + qm * LPm
    kl = T1 - T2

    l_z = np.take_along_axis(logits, (z_t % VOCAB)[..., None], axis=2)[..., 0]
    l_z = l_z.astype(np.float64)
    LPz = np.where(is_mask > 0.5, LPm, la + np.log(np.exp(l_z) + D) - logZ - logC)
    LQz = np.where(is_mask > 0.5, np.log(qm),
                   np.where(is_x > 0.5, np.log(qx), np.log(qc)))
    lr = LQz - LPz
    corr = -lr + np.exp(lr)
    lt = kl + corr

    mask = attention_mask.astype(np.float64)
    elbo = (elbo_w * lt + alpha_ratio).astype(np.float32)
    loss = np.float32((ws * lt * mask).sum() / mask.sum())
    if _trace:
        return (loss, elbo), res
    return (loss, elbo)


# revision 16
# speedup vs baseline: 1.3599x; 1.0124x over previous
"""GIDD loss kernel for Trainium2 (8 NeuronCores, token-parallel).

Math: with gamma=1 the q_t distribution takes only 3 distinct values per row
(at input_id, at MASK, elsewhere), so the vocab-wide KL reduces to two row
statistics computed on device:
    Z  = sum_{v != MASK} exp(logit_v)                  (softmax denominator)
    S  = sum_{v != MASK} log(exp(logit_v) + D),  D = (c_t/alpha_hat) * Z
Everything else is O(B*T) and is evaluated on the host while unsharding.

Device kernel per core: 256 tokens (2 tiles x 128 partitions), vocab on the
free dim. Pass 1: DMA logits chunks, ACT exp with fused accum (row sum -> Z).
exp values cached in SBUF as bf16. Pass 2: ACT ln(E + D) with per-partition
bias and fused accum (-> S). All tokens of one core belong to one sample, so
the single per-core scalar r = c_t/alpha_hat arrives as a tiny input tensor.
"""

import numpy as np

VOCAB = 32000
MASK_ID = 31999
NV = VOCAB - 1  # vocab columns participating in softmax/KL (mask col excluded)
B, T = 2, 1024
N_TOK = B * T
N_CORES = 8
TOK_PER_CORE = N_TOK // N_CORES  # 256
P = 128
N_TILES = TOK_PER_CORE // P  # 2
CHUNK = 6400
LAND_BUFS = 4
LOG_B = -11.0

_CACHE = {}


def _ensure_ntff_hook():
    """Provide antenv.axon_hooks (absent on this image) and install the
    NTFF profile hook so trace=True can report HW exec time."""
    import sys
    import types

    if "antenv.axon_hooks" not in sys.modules:
        mod = types.ModuleType("antenv.axon_hooks")
        mod._hook = None

        def set_axon_ntff_profile_hook(h, _mod=mod):
            _mod._hook = h

        def get_axon_ntff_profile_hook(_mod=mod):
            return _mod._hook

        mod.set_axon_ntff_profile_hook = set_axon_ntff_profile_hook
        mod.get_axon_ntff_profile_hook = get_axon_ntff_profile_hook
        sys.modules["antenv.axon_hooks"] = mod
        try:
            import antenv

            antenv.axon_hooks = mod
        except ImportError:
            pass
    try:
        from antenv.axon_hooks import (
            get_axon_ntff_profile_hook,
            set_axon_ntff_profile_hook,
        )

        if get_axon_ntff_profile_hook() is None:
            from trn_agent_boot.trn_boot import _ntff_profile_via_ctypes

            hook = _ntff_profile_via_ctypes("/opt/axon/libaxon_pjrt.so")
            if hook is not None:
                set_axon_ntff_profile_hook(hook)
    except Exception:
        pass


def _chunks(first_tile=False):
    # Stagger the leading chunks of the very first tile so the first exp
    # starts as soon as possible (time-to-first-chunk is paced by striped
    # DMA queues sharing HBM bandwidth).
    sizes = [1600, 1600, 3200] if first_tile else []
    o = sum(sizes)
    while o < NV:
        sizes.append(min(CHUNK, NV - o))
        o += sizes[-1]
    offs = []
    o = 0
    for s in sizes:
        offs.append((o, s))
        o += s
    return offs


def _force_single_act_table(bacc_mod):
    """Make the act-table pass satisfy Exp/Ln only via the combined
    natural_log_exp_and_others set, so the kernel loads one table once
    instead of ping-ponging between the exp-only and ln-only sets."""
    import bass_rust
    from concourse import mybir
    from concourse.hw_specs import get_activation_tables

    def patched(self):
        has_activation = any(
            isinstance(i, mybir.InstActivation)
            for b in self.main_func.blocks
            for i in b.instructions
        )
        if not has_activation:
            return
        both = {mybir.ActivationFunctionType.Exp,
                mybir.ActivationFunctionType.Ln}
        tables = []
        for name, funcs in get_activation_tables(self.m.arch).items():
            if name != "natural_log_exp_and_others":
                funcs = set(funcs) - both
            tables.append((name, funcs))
        bass_rust.insert_act_table_loads(self, tables)

    return patched


def _build():
    import concourse.tile as tile
    from concourse import bacc, mybir

    nc = bacc.Bacc("TRN2", target_bir_lowering=False, debug=False,
                   enable_asserts=False, num_devices=N_CORES)
    nc.insert_act_table_loads = _force_single_act_table(bacc).__get__(nc)
    xt = nc.dram_tensor("xt", [TOK_PER_CORE, VOCAB], mybir.dt.float16,
                        kind="ExternalInput")
    rconst = nc.dram_tensor("rconst", [P, 1], mybir.dt.float32,
                            kind="ExternalInput")
    out = nc.dram_tensor("out", [P, 2 * N_TILES], mybir.dt.float32,
                         kind="ExternalOutput")

    f32 = mybir.dt.float32
    f16 = mybir.dt.float16
    EXP = mybir.ActivationFunctionType.Exp
    LN = mybir.ActivationFunctionType.Ln

    with tile.TileContext(nc) as tc:
        with (
            tc.tile_pool(name="land", bufs=LAND_BUFS) as land_pool,
            tc.tile_pool(name="epool", bufs=N_TILES) as epool,
            tc.tile_pool(name="small", bufs=N_TILES) as small,
            tc.tile_pool(name="singles", bufs=1) as singles,
        ):
            rt = singles.tile([P, 1], f32)
            nc.sync.dma_start(out=rt, in_=rconst[:, :])

            for j in range(N_TILES):
                offs = _chunks(first_tile=(j == 0))
                nch = len(offs)
                E = epool.tile([P, NV], f16, tag="E")
                zparts = small.tile([P, 8], f32, tag="zparts")
                r0 = j * P
                for k, (off, cs) in enumerate(offs):
                    land = land_pool.tile([P, CHUNK], f16, tag="land")
                    nc.sync.dma_start(
                        out=land[:, :cs],
                        in_=xt[r0:r0 + P, off:off + cs],
                    )
                    nc.scalar.activation(
                        out=E[:, off:off + cs],
                        in_=land[:, :cs],
                        func=EXP,
                        accum_out=zparts[:, k:k + 1],
                    )
                Z1 = small.tile([P, 1], f32, tag="Z1")
                nc.vector.reduce_sum(Z1, zparts[:, :nch],
                                     axis=mybir.AxisListType.X)
                D = small.tile([P, 1], f32, tag="D")
                nc.vector.tensor_mul(D, Z1, rt)

                # single whole-row ln pass, in place over E, sum via accum
                S1 = small.tile([P, 1], f32, tag="S1")
                nc.scalar.activation(
                    out=E[:, :],
                    in_=E[:, :],
                    func=LN,
                    bias=D,
                    accum_out=S1,
                )

                st = small.tile([P, 2], f32, tag="st")
                nc.vector.tensor_copy(st[:, 0:1], Z1)
                nc.vector.tensor_copy(st[:, 1:2], S1)
                nc.sync.dma_start(out=out[:, 2 * j:2 * j + 2], in_=st)
    nc.compile()
    return nc


def _get_nc():
    if "nc" not in _CACHE:
        _CACHE["nc"] = _build()
    return _CACHE["nc"]


def _run_device(logits_flat, r_per_core, trace=False):
    _ensure_ntff_hook()
    from concourse.bass_utils import run_bass_kernel_spmd

    nc = _get_nc()
    in_maps = []
    for c in range(N_CORES):
        sl = logits_flat[c * TOK_PER_CORE:(c + 1) * TOK_PER_CORE]
        in_maps.append({
            "xt": np.ascontiguousarray(sl, dtype=np.float16),
            "rconst": np.full((P, 1), r_per_core[c], dtype=np.float32),
        })
    res = run_bass_kernel_spmd(nc, in_maps, list(range(N_CORES)), trace=trace)
    Z1 = np.empty(N_TOK, np.float64)
    S1 = np.empty(N_TOK, np.float64)
    for c in range(N_CORES):
        o = res.results[c]["out"]  # [P, 2*N_TILES]
        for j in range(N_TILES):
            t0 = c * TOK_PER_CORE + j * P
            Z1[t0:t0 + P] = o[:, 2 * j].astype(np.float64)
            S1[t0:t0 + P] = o[:, 2 * j + 1].astype(np.float64)
    return Z1.reshape(B, T), S1.reshape(B, T), res


def kernel(logits, input_ids, attention_mask, z_t, t, _trace=False):
    logits = np.asarray(logits)
    input_ids = np.asarray(input_ids)
    attention_mask = np.asarray(attention_mask)
    z_t = np.asarray(z_t)
    t = np.asarray(t)

    # ---- per-sample schedule scalars (float64 host math) ----
    t64 = t.astype(np.float64)  # [B]
    B_ = np.exp(np.float64(LOG_B))
    c = np.sqrt(t64) * np.sqrt(1.0 - t64) * B_
    C = 1.0 + (VOCAB - 2) * c
    alpha_hat = (1.0 - t64) - c
    c_p = 0.5 * (1.0 - 2.0 * t64) / (t64 * (1.0 - t64)) * c
    C_p = (VOCAB - 2) * c_p
    alpha_hat_p = -1.0 - c_p

    # ---- device: Z and S row statistics over the vocab ----
    r_tok = (c / alpha_hat)  # [B]
    core_b = (np.arange(N_CORES) * TOK_PER_CORE) // T
    r_per_core = r_tok[core_b].astype(np.float32)
    Z1, S1, res = _run_device(
        np.ascontiguousarray(logits.reshape(N_TOK, VOCAB)), r_per_core,
        trace=_trace)

    # ---- host: per-token weights ----
    tb = t64[:, None]  # [B,1]
    cb, Cb, ab = c[:, None], C[:, None], alpha_hat[:, None]
    cpb, Cpb, apb = c_p[:, None], C_p[:, None], alpha_hat_p[:, None]
    is_mask = (z_t == MASK_ID).astype(np.float64)
    pi_hat = tb * is_mask + cb * (1.0 - is_mask)
    pi_hat_p = 1.0 * is_mask + cpb * (1.0 - is_mask)
    alpha = ab / Cb
    pi_beta = pi_hat / Cb
    alpha_ratio = apb / ab - Cpb / Cb  # [B,1]
    omega = (pi_hat_p - apb / ab * pi_hat) / Cb
    is_x = (z_t == input_ids).astype(np.float64)
    elbo_w = (1.0 - is_x) * (omega / pi_beta) + is_x * (omega / (alpha + pi_beta))
    ws = np.clip(elbo_w, 0.0, 100.0)

    # ---- host: assemble KL + correction from device stats ----
    la = np.log(alpha_hat)[:, None]
    logC = np.log(C)[:, None]
    logZ = np.log(Z1)  # [B,T]
    D = (c / alpha_hat)[:, None] * Z1
    # log p_t[v != MASK] = la + log(exp(l_v) + D) - logZ - logC
    sumLP_nm = NV * (la - logZ - logC) + S1
    l_x = np.take_along_axis(logits, input_ids[..., None], axis=2)[..., 0]
    l_x = l_x.astype(np.float64)
    LPx = la + np.log(np.exp(l_x) + D) - logZ - logC
    LPm = np.log(tb) - logC
    qc = cb / Cb
    qx = (ab + cb) / Cb
    qm = tb / Cb
    T1 = (VOCAB - 2) * qc * np.log(qc) + qx * np.log(qx) + qm * np.log(qm)
    T2 = qc * (sumLP_nm - LPx) + qx * LPx + qm * LPm
    kl = T1 - T2

    l_z = np.take_along_axis(logits, (z_t % VOCAB)[..., None], axis=2)[..., 0]
    l_z = l_z.astype(np.float64)
    LPz = np.where(is_mask > 0.5, LPm, la + np.log(np.exp(l_z) + D) - logZ - logC)
    LQz = np.where(is_mask > 0.5, np.log(qm),
                   np.where(is_x > 0.5, np.log(qx), np.log(qc)))
    lr = LQz - LPz
    corr = -lr + np.exp(lr)
    lt = kl + corr

    mask = attention_mask.astype(np.float64)
    elbo = (elbo_w * lt + alpha_ratio).astype(np.float32)
    loss = np.float32((ws * lt * mask).sum() / mask.sum())
    if _trace:
        return (loss, elbo), res
    return (loss, elbo)


# revision 20
# speedup vs baseline: 1.3802x; 1.0149x over previous
"""GIDD loss kernel for Trainium2 (8 NeuronCores, token-parallel).

Math: with gamma=1 the q_t distribution takes only 3 distinct values per row
(at input_id, at MASK, elsewhere), so the vocab-wide KL reduces to two row
statistics computed on device:
    Z  = sum_{v != MASK} exp(logit_v)                  (softmax denominator)
    S  = sum_{v != MASK} log(exp(logit_v) + D),  D = (c_t/alpha_hat) * Z
Everything else is O(B*T) and is evaluated on the host while unsharding.

Device kernel per core: 256 tokens (2 tiles x 128 partitions), vocab on the
free dim. Pass 1: DMA logits chunks, ACT exp with fused accum (row sum -> Z).
exp values cached in SBUF as bf16. Pass 2: ACT ln(E + D) with per-partition
bias and fused accum (-> S). All tokens of one core belong to one sample, so
the single per-core scalar r = c_t/alpha_hat arrives as a tiny input tensor.
"""

import numpy as np

VOCAB = 32000
MASK_ID = 31999
NV = VOCAB - 1  # vocab columns participating in softmax/KL (mask col excluded)
B, T = 2, 1024
N_TOK = B * T
N_CORES = 8
TOK_PER_CORE = N_TOK // N_CORES  # 256
P = 128
N_TILES = TOK_PER_CORE // P  # 2
CHUNK = 6400
LAND_BUFS = 4
LOG_B = -11.0

_CACHE = {}


def _ensure_ntff_hook():
    """Provide antenv.axon_hooks (absent on this image) and install the
    NTFF profile hook so trace=True can report HW exec time."""
    import sys
    import types

    if "antenv.axon_hooks" not in sys.modules:
        mod = types.ModuleType("antenv.axon_hooks")
        mod._hook = None

        def set_axon_ntff_profile_hook(h, _mod=mod):
            _mod._hook = h

        def get_axon_ntff_profile_hook(_mod=mod):
            return _mod._hook

        mod.set_axon_ntff_profile_hook = set_axon_ntff_profile_hook
        mod.get_axon_ntff_profile_hook = get_axon_ntff_profile_hook
        sys.modules["antenv.axon_hooks"] = mod
        try:
            import antenv

            antenv.axon_hooks = mod
        except ImportError:
            pass
    try:
        from antenv.axon_hooks import (
            get_axon_ntff_profile_hook,
            set_axon_ntff_profile_hook,
        )

        if get_axon_ntff_profile_hook() is None:
            from trn_agent_boot.trn_boot import _ntff_profile_via_ctypes

            hook = _ntff_profile_via_ctypes("/opt/axon/libaxon_pjrt.so")
            if hook is not None:
                set_axon_ntff_profile_hook(hook)
    except Exception:
        pass


def _chunks(first_tile=False):
    # Stagger the leading chunks of the very first tile so the first exp
    # starts as soon as possible (time-to-first-chunk is paced by striped
    # DMA queues sharing HBM bandwidth).
    sizes = [1600, 1600, 1600, 3200, 4800] if first_tile else []
    o = sum(sizes)
    while o < NV:
        sizes.append(min(CHUNK, NV - o))
        o += sizes[-1]
    offs = []
    o = 0
    for s in sizes:
        offs.append((o, s))
        o += s
    return offs


def _force_single_act_table(bacc_mod):
    """Make the act-table pass satisfy Exp/Ln only via the combined
    natural_log_exp_and_others set, so the kernel loads one table once
    instead of ping-ponging between the exp-only and ln-only sets."""
    import bass_rust
    from concourse import mybir
    from concourse.hw_specs import get_activation_tables

    def patched(self):
        has_activation = any(
            isinstance(i, mybir.InstActivation)
            for b in self.main_func.blocks
            for i in b.instructions
        )
        if not has_activation:
            return
        both = {mybir.ActivationFunctionType.Exp,
                mybir.ActivationFunctionType.Ln}
        tables = []
        for name, funcs in get_activation_tables(self.m.arch).items():
            if name != "natural_log_exp_and_others":
                funcs = set(funcs) - both
            tables.append((name, funcs))
        bass_rust.insert_act_table_loads(self, tables)

    return patched


def _build():
    import concourse.tile as tile
    from concourse import bacc, mybir

    nc = bacc.Bacc("TRN2", target_bir_lowering=False, debug=False,
                   enable_asserts=False, num_devices=N_CORES)
    nc.insert_act_table_loads = _force_single_act_table(bacc).__get__(nc)
    xt = nc.dram_tensor("xt", [TOK_PER_CORE, VOCAB], mybir.dt.float16,
                        kind="ExternalInput")
    rconst = nc.dram_tensor("rconst", [P, 1], mybir.dt.float32,
                            kind="ExternalInput")
    out = nc.dram_tensor("out", [P, 2 * N_TILES], mybir.dt.float32,
                         kind="ExternalOutput")

    f32 = mybir.dt.float32
    f16 = mybir.dt.float16
    EXP = mybir.ActivationFunctionType.Exp
    LN = mybir.ActivationFunctionType.Ln

    with tile.TileContext(nc) as tc:
        with (
            tc.tile_pool(name="land", bufs=LAND_BUFS) as land_pool,
            tc.tile_pool(name="epool", bufs=N_TILES) as epool,
            tc.tile_pool(name="small", bufs=N_TILES) as small,
            tc.tile_pool(name="singles", bufs=1) as singles,
        ):
            rt = singles.tile([P, 1], f32)
            rt_loaded = False

            for j in range(N_TILES):
                offs = _chunks(first_tile=(j == 0))
                nch = len(offs)
                E = epool.tile([P, NV], f16, tag="E")
                zparts = small.tile([P, 8], f32, tag="zparts")
                r0 = j * P
                for k, (off, cs) in enumerate(offs):
                    land = land_pool.tile([P, CHUNK], f16, tag="land")
                    nc.sync.dma_start(
                        out=land[:, :cs],
                        in_=xt[r0:r0 + P, off:off + cs],
                    )
                    nc.scalar.activation(
                        out=E[:, off:off + cs],
                        in_=land[:, :cs],
                        func=EXP,
                        accum_out=zparts[:, k:k + 1],
                    )
                    if not rt_loaded:
                        # issue after the first logits chunk so it doesn't
                        # delay the critical first DMA
                        nc.sync.dma_start(out=rt, in_=rconst[:, :])
                        rt_loaded = True
                Z1 = small.tile([P, 1], f32, tag="Z1")
                nc.vector.reduce_sum(Z1, zparts[:, :nch],
                                     axis=mybir.AxisListType.X)
                D = small.tile([P, 1], f32, tag="D")
                nc.vector.tensor_mul(D, Z1, rt)

                # single whole-row ln pass, in place over E, sum via accum
                S1 = small.tile([P, 1], f32, tag="S1")
                nc.scalar.activation(
                    out=E[:, :],
                    in_=E[:, :],
                    func=LN,
                    bias=D,
                    accum_out=S1,
                )

                nc.sync.dma_start(out=out[:, 2 * j:2 * j + 1], in_=Z1)
                nc.sync.dma_start(out=out[:, 2 * j + 1:2 * j + 2], in_=S1)
    nc.compile()
    return nc


def _get_nc():
    if "nc" not in _CACHE:
        _CACHE["nc"] = _build()
    return _CACHE["nc"]


def _run_device(logits_flat, r_per_core, trace=False):
    _ensure_ntff_hook()
    from concourse.bass_utils import run_bass_kernel_spmd

    nc = _get_nc()
    in_maps = []
    for c in range(N_CORES):
        sl = logits_flat[c * TOK_PER_CORE:(c + 1) * TOK_PER_CORE]
        in_maps.append({
            "xt": np.ascontiguousarray(sl, dtype=np.float16),
            "rconst": np.full((P, 1), r_per_core[c], dtype=np.float32),
        })
    res = run_bass_kernel_spmd(nc, in_maps, list(range(N_CORES)), trace=trace)
    Z1 = np.empty(N_TOK, np.float64)
    S1 = np.empty(N_TOK, np.float64)
    for c in range(N_CORES):
        o = res.results[c]["out"]  # [P, 2*N_TILES]
        for j in range(N_TILES):
            t0 = c * TOK_PER_CORE + j * P
            Z1[t0:t0 + P] = o[:, 2 * j].astype(np.float64)
            S1[t0:t0 + P] = o[:, 2 * j + 1].astype(np.float64)
    return Z1.reshape(B, T), S1.reshape(B, T), res


def kernel(logits, input_ids, attention_mask, z_t, t, _trace=False):
    logits = np.asarray(logits)
    input_ids = np.asarray(input_ids)
    attention_mask = np.asarray(attention_mask)
    z_t = np.asarray(z_t)
    t = np.asarray(t)

    # ---- per-sample schedule scalars (float64 host math) ----
    t64 = t.astype(np.float64)  # [B]
    B_ = np.exp(np.float64(LOG_B))
    c = np.sqrt(t64) * np.sqrt(1.0 - t64) * B_
    C = 1.0 + (VOCAB - 2) * c
    alpha_hat = (1.0 - t64) - c
    c_p = 0.5 * (1.0 - 2.0 * t64) / (t64 * (1.0 - t64)) * c
    C_p = (VOCAB - 2) * c_p
    alpha_hat_p = -1.0 - c_p

    # ---- device: Z and S row statistics over the vocab ----
    r_tok = (c / alpha_hat)  # [B]
    core_b = (np.arange(N_CORES) * TOK_PER_CORE) // T
    r_per_core = r_tok[core_b].astype(np.float32)
    Z1, S1, res = _run_device(
        np.ascontiguousarray(logits.reshape(N_TOK, VOCAB)), r_per_core,
        trace=_trace)

    # ---- host: per-token weights ----
    tb = t64[:, None]  # [B,1]
    cb, Cb, ab = c[:, None], C[:, None], alpha_hat[:, None]
    cpb, Cpb, apb = c_p[:, None], C_p[:, None], alpha_hat_p[:, None]
    is_mask = (z_t == MASK_ID).astype(np.float64)
    pi_hat = tb * is_mask + cb * (1.0 - is_mask)
    pi_hat_p = 1.0 * is_mask + cpb * (1.0 - is_mask)
    alpha = ab / Cb
    pi_beta = pi_hat / Cb
    alpha_ratio = apb / ab - Cpb / Cb  # [B,1]
    omega = (pi_hat_p - apb / ab * pi_hat) / Cb
    is_x = (z_t == input_ids).astype(np.float64)
    elbo_w = (1.0 - is_x) * (omega / pi_beta) + is_x * (omega / (alpha + pi_beta))
    ws = np.clip(elbo_w, 0.0, 100.0)

    # ---- host: assemble KL + correction from device stats ----
    la = np.log(alpha_hat)[:, None]
    logC = np.log(C)[:, None]
    logZ = np.log(Z1)  # [B,T]
    D = (c / alpha_hat)[:, None] * Z1
    # log p_t[v != MASK] = la + log(exp(l_v) + D) - logZ - logC
    sumLP_nm = NV * (la - logZ - logC) + S1
    l_x = np.take_along_axis(logits, input_ids[..., None], axis=2)[..., 0]
    l_x = l_x.astype(np.float64)
    LPx = la + np.log(np.exp(l_x) + D) - logZ - logC
    LPm = np.log(tb) - logC
    qc = cb / Cb
    qx = (ab + cb) / Cb
    qm = tb / Cb
    T1 = (VOCAB - 2) * qc * np.log(qc) + qx * np.log(qx) + qm * np.log(qm)
    T2 = qc * (sumLP_nm - LPx) + qx * LPx + qm * LPm
    kl = T1 - T2

    l_z = np.take_along_axis(logits, (z_t % VOCAB)[..., None], axis=2)[..., 0]
    l_z = l_z.astype(np.float64)
    LPz = np.where(is_mask > 0.5, LPm, la + np.log(np.exp(l_z) + D) - logZ - logC)
    LQz = np.where(is_mask > 0.5, np.log(qm),
                   np.where(is_x > 0.5, np.log(qx), np.log(qc)))
    lr = LQz - LPz
    corr = -lr + np.exp(lr)
    lt = kl + corr

    mask = attention_mask.astype(np.float64)
    elbo = (elbo_w * lt + alpha_ratio).astype(np.float32)
    loss = np.float32((ws * lt * mask).sum() / mask.sum())
    if _trace:
        return (loss, elbo), res
    return (loss, elbo)
